# revision 1
# baseline (speedup 1.0000x reference)
from contextlib import ExitStack

import numpy as np
import ml_dtypes

import concourse.bacc as bacc
import concourse.mybir as mybir
import concourse.tile as tile
from concourse.bass_utils import run_bass_kernel_spmd

B, C, H, Wd = 4, 2048, 64, 64
HW = H * Wd
HID = 256
P = 128
CC = C // P
OC = HID // P
PT = HW // P
CT = C // P
NCORES = 8
QH = 2048
QSUPER = 512
NSUPER = QH // QSUPER
QB = 512
SHIFT = 100.0
WARMUP_MMS = 40
POOL_MODE = "queue"
EXPP_BUFS = 2
XPIN_BUFS = 4
A_EMIT_CTH = 1
STPS_BUFS = 3
UPS_BUFS = 2

F32 = mybir.dt.float32
F32R = mybir.dt.float32r
BF16 = mybir.dt.bfloat16
F16 = mybir.dt.float16
PROJ_F16 = False

_CACHE: dict = {}


def _build():
    nc = bacc.Bacc()
    pdt = F16 if PROJ_F16 else F32R
    xq = nc.declare_dram_parameter("xq", [C, QH], pdt, isOutput=False)
    xp = nc.declare_dram_parameter("xp", [C, HW], pdt, isOutput=False)
    xpt = nc.declare_dram_parameter("xpt", [HW, C], BF16, isOutput=False)
    wt = nc.declare_dram_parameter("wt", [C, HID], pdt, isOutput=False)
    bias = nc.declare_dram_parameter("bias", [HID], F32, isOutput=False)
    out = nc.declare_dram_parameter("out", [C, QH], F32, isOutput=True)

    with tile.TileContext(nc, pool_alloc_mode=POOL_MODE) as tc:
        with (
            tc.tile_pool(name="proj", bufs=1) as proj_pool,
            tc.tile_pool(name="misc", bufs=1) as misc_pool,
            tc.tile_pool(name="wtp", bufs=1) as wtp,
            tc.tile_pool(name="xqin", bufs=2) as xqinp,
            tc.tile_pool(name="expp", bufs=EXPP_BUFS) as expp,
            tc.tile_pool(name="lone", bufs=1) as lonep,
            tc.tile_pool(name="pps", bufs=2, space="PSUM") as pps,
            tc.tile_pool(name="stps", bufs=STPS_BUFS, space="PSUM") as stps,
            tc.tile_pool(name="ups", bufs=UPS_BUFS, space="PSUM") as ups,
            tc.tile_pool(name="bcps", bufs=1, space="PSUM") as bcps,
        ):
            kproj = proj_pool.tile([P, OC, HW], F32R)
            qproj = proj_pool.tile([P, OC, QH], F32R)
            bias_sb = misc_pool.tile([P, OC], F32)
            nc.sync.dma_start(bias_sb[:], bias.rearrange("(oc p) -> p oc", p=P))
            ones_row = misc_pool.tile([1, P], F32)
            nc.gpsimd.memset(ones_row[:], 1.0)
            ones_col = misc_pool.tile([P, 1], F32)
            nc.gpsimd.memset(ones_col[:], 1.0)
            neg_shift = misc_pool.tile([P, 1], F32)
            nc.gpsimd.memset(neg_shift[:], -SHIFT)

            wu = misc_pool.tile([P, QB], BF16)
            nc.gpsimd.memset(wu[:], 0.0)
            wu_ps = pps.tile([P, QB], F32, tag="pp")
            for _ in range(WARMUP_MMS):
                nc.tensor.matmul(wu_ps[:], wu[:, :P], wu[:], start=True, stop=True)

            wt_r = wt.rearrange("(cc p) o -> p cc o", p=P)
            wt_sb = wtp.tile([P, CC, HID], pdt)
            nc.sync.dma_start(wt_sb[:, :CC // 2], wt_r[:, :CC // 2])
            nc.sync.dma_start(wt_sb[:, CC // 2:], wt_r[:, CC // 2:])

            def proj_block(src, dst, blk, pool):
                src_r = src.rearrange("(cc p) n -> p cc n", p=P)
                nq = 4
                quarters = []
                for h in range(nq):
                    xin = pool.tile([P, CC // nq, QB], pdt, tag="xin")
                    nc.sync.dma_start(
                        xin[:],
                        src_r[:, h * (CC // nq):(h + 1) * (CC // nq),
                              blk * QB:(blk + 1) * QB],
                    )
                    quarters.append(xin)
                for ot in range(OC):
                    ps = pps.tile([P, QB], F32, tag="pp")
                    for k in range(CC):
                        nc.tensor.matmul(
                            ps[:],
                            wt_sb[:, k, ot * P:(ot + 1) * P],
                            quarters[k // (CC // nq)][:, k % (CC // nq), :],
                            start=(k == 0),
                            stop=(k == CC - 1),
                        )
                    nc.vector.tensor_scalar_add(
                        dst[:, ot, blk * QB:(blk + 1) * QB],
                        ps[:],
                        bias_sb[:, ot:ot + 1],
                    )

            def stage_a_tile(pt, qs, lacc):
                st = stps.tile([P, QSUPER], F32, tag="st")
                for oc_i in range(OC):
                    nc.tensor.matmul(
                        st[:],
                        kproj[:, oc_i, pt * P:(pt + 1) * P],
                        qproj[:, oc_i, qs:qs + QSUPER],
                        start=(oc_i == 0),
                        stop=(oc_i == OC - 1),
                    )
                et = expp.tile([P, QSUPER], BF16, tag=f"expT{pt}")
                nc.scalar.activation(
                    et[:], st[:],
                    mybir.ActivationFunctionType.Exp,
                    bias=neg_shift[:],
                )
                if pt == 0:
                    nc.vector.tensor_copy(lacc[:], et[:])
                else:
                    nc.vector.tensor_add(out=lacc[:], in0=lacc[:], in1=et[:])
                return et

            def stage_a(qs, la_slot):
                la = lonep.tile([P, QSUPER], F32, tag=f"ltree{la_slot}")
                return [stage_a_tile(pt, qs, la) for pt in range(PT)], la

            xpt_r = xpt.rearrange("(pt p) c -> p pt c", p=P)
            ph2 = ExitStack()
            xv0p = ph2.enter_context(tc.tile_pool(name="xv0p", bufs=1))
            xv0 = xv0p.tile([P, PT, 2 * P], BF16)
            proj_block(xq, qproj, 0, xqinp)
            nc.sync.dma_start(xv0[:], xpt_r[:, :, 0:2 * P])
            expT0 = []
            la0 = lonep.tile([P, QSUPER], F32, tag="ltree0")
            up01 = [ups.tile([P, QSUPER], F32, tag="u", name=f"up0{i}")
                    for i in range(2)]
            with tc.tile_pool(name="xpin", bufs=XPIN_BUFS) as xpinp:
                for blk in range(HW // QB):
                    proj_block(xp, kproj, blk, xpinp)
                    pts = range(QB // P * blk, QB // P * (blk + 1))
                    for pt in pts:
                        expT0.append(stage_a_tile(pt, 0, la0))
                    for ci in range(2):
                        for pt in pts:
                            nc.tensor.matmul(
                                up01[ci][:],
                                xv0[:, pt, ci * P:(ci + 1) * P],
                                expT0[pt][:],
                                start=(pt == 0),
                                stop=(pt == PT - 1),
                            )

            with ph2:
                xvp = ph2.enter_context(tc.tile_pool(name="xvp", bufs=2))
                osbp = ph2.enter_context(tc.tile_pool(name="osbp", bufs=2))
                bcsbp = ph2.enter_context(tc.tile_pool(name="bcsb", bufs=2))

                state = {"expT": expT0, "la": la0}
                for s in range(NSUPER):
                    qs = s * QSUPER
                    expT, la = state["expT"], state["la"]
                    bt = None
                    pending = []
                    groups_done = 0
                    for cth in range(CT // 2):
                        if s == 0 and cth == 0:
                            xv = xv0
                        else:
                            xv = xvp.tile([P, PT, 2 * P], BF16, tag="xv")
                            nc.sync.dma_start(
                                xv[:], xpt_r[:, :, cth * 2 * P:(cth + 1) * 2 * P]
                            )
                        for ci in range(2):
                            ct = cth * 2 + ci
                            osb = osbp.tile([P, QSUPER], F32, tag="osb")
                            if s == 0 and cth == 0:
                                up = up01[ci]
                            else:
                                up = ups.tile([P, QSUPER], F32, tag="u")
                                for pt in range(PT):
                                    nc.tensor.matmul(
                                        up[:],
                                        xv[:, pt, ci * P:(ci + 1) * P],
                                        expT[pt][:],
                                        start=(pt == 0),
                                        stop=(pt == PT - 1),
                                    )
                            groups_done += 1
                            if bt is None and groups_done >= 2:
                                lp = bcps.tile([1, QSUPER], F32, tag="bc")
                                nc.tensor.matmul(
                                    lp[:], ones_col[:], la[:],
                                    start=True, stop=True,
                                )
                                l_sb = lonep.tile([1, QSUPER], F32, tag="lsb")
                                nc.vector.tensor_copy(l_sb[:], lp[:])
                                bcp = bcps.tile([P, QSUPER], F32, tag="bc")
                                nc.tensor.matmul(
                                    bcp[:], ones_row[:], l_sb[:],
                                    start=True, stop=True,
                                )
                                bt = bcsbp.tile([P, QSUPER], F32, tag="bcr")
                                nc.vector.reciprocal(bt[:], bcp[:])
                                for posb, pup, pct in pending:
                                    nc.vector.tensor_mul(
                                        out=posb[:], in0=pup[:], in1=bt[:]
                                    )
                                    nc.sync.dma_start(
                                        out[pct * P:(pct + 1) * P,
                                            qs:qs + QSUPER],
                                        posb[:],
                                    )
                                pending = []
                            if bt is None:
                                pending.append((osb, up, ct))
                                continue
                            nc.vector.tensor_mul(out=osb[:], in0=up[:], in1=bt[:])
                            nc.sync.dma_start(
                                out[ct * P:(ct + 1) * P, qs:qs + QSUPER], osb[:]
                            )
                        if cth == A_EMIT_CTH and s + 1 < NSUPER:
                            proj_block(xq, qproj, s + 1, xqinp)
                            nexpT, nla = stage_a(
                                (s + 1) * QSUPER, (s + 1) % 2
                            )
                            state = {"expT": nexpT, "la": nla}
    nc.finalize()
    return nc


def _get_nc():
    if "nc" not in _CACHE:
        _CACHE["nc"] = _build()
    return _CACHE["nc"]


def _make_in_maps(query_features, prompt_features, W, b):
    qf = np.asarray(query_features, dtype=np.float32)
    pf = np.asarray(prompt_features, dtype=np.float32)
    Wm = np.asarray(W, dtype=np.float32)
    bv = np.asarray(b, dtype=np.float32)

    pnp = np.float16 if PROJ_F16 else np.float32
    wt = np.ascontiguousarray(Wm.T).astype(pnp)
    xps = [np.ascontiguousarray(pf[bi].reshape(C, HW)) for bi in range(B)]
    xps_in = [x.astype(pnp) for x in xps]
    xpts = [
        np.ascontiguousarray(xps[bi].T).astype(ml_dtypes.bfloat16)
        for bi in range(B)
    ]
    in_maps = []
    for core in range(NCORES):
        bi, h = divmod(core, 2)
        xq = np.ascontiguousarray(qf[bi].reshape(C, HW)[:, h * QH:(h + 1) * QH]).astype(pnp)
        in_maps.append(
            {"xq": xq, "xp": xps_in[bi], "xpt": xpts[bi], "wt": wt, "bias": bv}
        )
    return in_maps


def _assemble(results):
    full = np.empty((B, C, HW), np.float32)
    for core in range(NCORES):
        bi, h = divmod(core, 2)
        full[bi][:, h * QH:(h + 1) * QH] = results[core]["out"]
    return full.reshape(B, C, H, Wd)


def kernel(query_features, prompt_features, W, b):
    nc = _get_nc()
    in_maps = _make_in_maps(query_features, prompt_features, W, b)
    last_err = None
    for _ in range(3):
        try:
            res = run_bass_kernel_spmd(nc, in_maps, list(range(NCORES)))
            return _assemble(res.results)
        except Exception as e:
            last_err = e
    raise last_err


def kernel_traced(query_features, prompt_features, W, b, **trace_kwargs):
    nc = _get_nc()
    in_maps = _make_in_maps(query_features, prompt_features, W, b)
    res = run_bass_kernel_spmd(
        nc, in_maps, list(range(NCORES)), trace=True, **trace_kwargs
    )
    return _assemble(res.results), res



# revision 8
# speedup vs baseline: 1.1045x; 1.1045x over previous
import numpy as np
import ml_dtypes

import concourse.bacc as bacc
import concourse.mybir as mybir
import concourse.tile as tile
from concourse.bass_utils import run_bass_kernel_spmd

B, C, H, Wd = 4, 2048, 64, 64
HW = H * Wd
HID = 256
P = 128
CC = C // P
OC = HID // P
PT = HW // P
NPAIR = PT // 2
CT = C // P
NCORES = 8
QH = 2048
QS = 256
NS = QH // QS
QB = 512
SHIFT = 100.0
CSCALE = 128.0
WARMUP_MMS = 14
POOL_MODE = "queue"
VL_CTG = 4

F32 = mybir.dt.float32
F16 = mybir.dt.float16
BF16 = mybir.dt.bfloat16
FP8 = mybir.dt.float8e4
DR = mybir.MatmulPerfMode.DoubleRow

_CACHE: dict = {}


def _build():
    nc = bacc.Bacc()
    xq = nc.declare_dram_parameter("xq", [C, QH], F16, isOutput=False)
    xp = nc.declare_dram_parameter("xp", [C, HW], F16, isOutput=False)
    vh = nc.declare_dram_parameter("vh", [HW, C], FP8, isOutput=False)
    vl = nc.declare_dram_parameter("vl", [HW, C], FP8, isOutput=False)
    wt = nc.declare_dram_parameter("wt", [C, HID], F16, isOutput=False)
    bias = nc.declare_dram_parameter("bias", [HID], F32, isOutput=False)
    out = nc.declare_dram_parameter("out", [C, QH], F32, isOutput=True)

    with tile.TileContext(nc, pool_alloc_mode=POOL_MODE) as tc:
        with (
            tc.tile_pool(name="proj", bufs=1) as proj_pool,
            tc.tile_pool(name="misc", bufs=1) as misc_pool,
            tc.tile_pool(name="wtp", bufs=1) as wtp,
            tc.tile_pool(name="vthp", bufs=1) as vthp,
            tc.tile_pool(name="vlp", bufs=2) as vlp,
            tc.tile_pool(name="xqin", bufs=2) as xqinp,
            tc.tile_pool(name="etp", bufs=1) as etp,
            tc.tile_pool(name="a8p", bufs=2) as a8p,
            tc.tile_pool(name="lonep", bufs=1) as lonep,
            tc.tile_pool(name="osbp", bufs=2) as osbp,
            tc.tile_pool(name="bcsb", bufs=2) as bcsbp,
            tc.tile_pool(name="pps", bufs=2, space="PSUM") as pps,
            tc.tile_pool(name="stps", bufs=3, space="PSUM") as stps,
            tc.tile_pool(name="ups", bufs=2, space="PSUM") as ups,
            tc.tile_pool(name="bcps", bufs=1, space="PSUM") as bcps,
        ):
            kproj = proj_pool.tile([P, OC, HW], F16)
            qproj = proj_pool.tile([P, OC, QH], F16)
            bias_sb = misc_pool.tile([P, OC], F32)
            nc.sync.dma_start(bias_sb[:], bias.rearrange("(oc p) -> p oc", p=P))
            ones_col = misc_pool.tile([P, 1], BF16)
            nc.gpsimd.memset(ones_col[:], 1.0)
            ones_row_c = misc_pool.tile([1, P], BF16)
            nc.gpsimd.memset(ones_row_c[:], 1.0 / CSCALE)
            ones_row_1 = misc_pool.tile([1, P], BF16)
            nc.gpsimd.memset(ones_row_1[:], 1.0)
            ones8 = misc_pool.tile([P, 2, 16], FP8)
            nc.gpsimd.memset(ones8[:], 1.0)
            neg_shift = misc_pool.tile([P, 1], F32)
            nc.gpsimd.memset(neg_shift[:], -SHIFT)

            wu = misc_pool.tile([P, QB], BF16)
            nc.gpsimd.memset(wu[:], 0.0)
            wu_ps = pps.tile([P, QB], F32, tag="pp")
            for _ in range(WARMUP_MMS):
                nc.tensor.matmul(wu_ps[:], wu[:, :P], wu[:], start=True, stop=True)

            wt_r = wt.rearrange("(cc p) o -> p cc o", p=P)
            wt_sb = wtp.tile([P, CC, HID], F16)
            nc.sync.dma_start(wt_sb[:], wt_r[:])

            NQ = 4

            def proj_dma(src, blk, pool):
                src_r = src.rearrange("(cc p) n -> p cc n", p=P)
                halves = []
                for h in range(NQ):
                    xin = pool.tile([P, CC // NQ, QB], F16, tag="xin",
                                    name=f"xin_{blk}_{h}")
                    nc.sync.dma_start(
                        xin[:],
                        src_r[:, h * (CC // NQ):(h + 1) * (CC // NQ),
                              blk * QB:(blk + 1) * QB],
                    )
                    halves.append(xin)
                return halves

            def proj_mms(dst, blk, halves):
                for ot in range(OC):
                    ps = pps.tile([P, QB], F32, tag="pp", name=f"pp_{blk}_{ot}")
                    for k in range(CC):
                        nc.tensor.matmul(
                            ps[:],
                            wt_sb[:, k, ot * P:(ot + 1) * P],
                            halves[k // (CC // NQ)][:, k % (CC // NQ), :],
                            start=(k == 0),
                            stop=(k == CC - 1),
                        )
                    nc.vector.tensor_scalar_add(
                        dst[:, ot, blk * QB:(blk + 1) * QB],
                        ps[:],
                        bias_sb[:, ot:ot + 1],
                    )

            def proj_block(src, dst, blk, pool):
                proj_mms(dst, blk, proj_dma(src, blk, pool))

            def stage_a_tile(pt, s, la):
                qs = s * QS
                st = stps.tile([P, QS], F32, tag="st", name=f"st_{s}_{pt}")
                for oc_i in range(OC):
                    nc.tensor.matmul(
                        st[:],
                        kproj[:, oc_i, pt * P:(pt + 1) * P],
                        qproj[:, oc_i, qs:qs + QS],
                        start=(oc_i == 0),
                        stop=(oc_i == OC - 1),
                    )
                et = etp.tile([P, QS], BF16, tag=f"et{pt}", name=f"et_{s}_{pt}")
                nc.scalar.activation(
                    et[:], st[:],
                    mybir.ActivationFunctionType.Exp,
                    bias=neg_shift[:],
                )
                if pt == 0:
                    nc.vector.tensor_copy(la[:], et[:])
                else:
                    nc.vector.tensor_add(out=la[:], in0=la[:], in1=et[:])
                return et

            def new_la(s):
                return lonep.tile([P, QS], BF16, tag=f"la{s % 2}", name=f"la_{s}")

            def c_chain(s, la_t):
                lp = bcps.tile([1, QS], F32, tag="bc", name=f"lp_{s}")
                nc.tensor.matmul(lp[:], ones_col[:], la_t[:], start=True, stop=True)
                l_sb = lonep.tile([1, QS], BF16, tag="lsb", name=f"lsb_{s}")
                nc.vector.tensor_copy(l_sb[:], lp[:])
                bcp = bcps.tile([P, QS], F32, tag="bc", name=f"bcp_{s}")
                nc.tensor.matmul(bcp[:], ones_row_c[:], l_sb[:], start=True, stop=True)
                cbt = bcsbp.tile([P, QS], BF16, tag="cbt", name=f"cbt_{s}")
                with nc.allow_low_precision(
                    reason="c is a per-row rescale; cancelled by l'' renorm"
                ):
                    nc.vector.reciprocal(cbt[:], bcp[:])
                return cbt

            def conv_pair(s, t, ets_s, cbt):
                ah = a8p.tile([P, 2, QS], FP8, tag=f"ah{t}", name=f"ah_{s}_{t}")
                al = a8p.tile([P, 2, QS], FP8, tag=f"al{t}", name=f"al_{s}_{t}")
                for i in range(2):
                    et = ets_s[2 * t + i]
                    nc.vector.tensor_mul(out=et[:], in0=et[:], in1=cbt[:])
                    nc.scalar.activation(
                        ah[:, i, :], et[:], mybir.ActivationFunctionType.Copy
                    )
                    nc.vector.tensor_sub(out=al[:, i, :], in0=et[:], in1=ah[:, i, :])
                return ah, al

            proj_block(xq, qproj, 0, xqinp)
            vh_r = vh.rearrange("(pt p) c -> p pt c", p=P)
            vth = vthp.tile([P, PT, C], FP8)
            la = {0: new_la(0)}
            ets = {0: []}
            with tc.tile_pool(name="xpin", bufs=2) as xpinp:
                for blk in range(HW // QB):
                    proj_block(xp, kproj, blk, xpinp)
                    for pt in range(QB // P * blk, QB // P * (blk + 1)):
                        ets[0].append(stage_a_tile(pt, 0, la[0]))
            nc.sync.dma_start(vth[:], vh_r[:])

            cbt0 = c_chain(0, la[0])
            a8 = {0: ([], [])}
            la[1] = new_la(1)
            ets[1] = []
            qp1 = proj_dma(xq, 1, xqinp)
            for t in range(NPAIR):
                ah, alo = conv_pair(0, t, ets[0], cbt0)
                a8[0][0].append(ah)
                a8[0][1].append(alo)
                if t == 2:
                    proj_mms(qproj, 1, qp1)
                if t >= 2:
                    for pt in (2 * (t - 2), 2 * (t - 2) + 1):
                        ets[1].append(stage_a_tile(pt, 1, la[1]))
            for pt in range(2 * (NPAIR - 2), PT):
                ets[1].append(stage_a_tile(pt, 1, la[1]))

            vl_r = vl.rearrange("(pt p) c -> p pt c", p=P)

            for s in range(NS):
                qs = s * QS
                ah_t, al_t = a8[s]
                l2p = bcps.tile([16, QS], F32, tag="bc", name=f"l2p_{s}")
                for t in range(NPAIR):
                    nc.tensor.matmul(
                        l2p[:], ones8[:], ah_t[t][:],
                        start=(t == 0), stop=False, perf_mode=DR,
                    )
                for t in range(NPAIR):
                    nc.tensor.matmul(
                        l2p[:], ones8[:], al_t[t][:],
                        start=False, stop=(t == NPAIR - 1), perf_mode=DR,
                    )
                l2_sb = lonep.tile([1, QS], BF16, tag="l2sb", name=f"l2sb_{s}")
                nc.vector.tensor_copy(l2_sb[:], l2p[0:1, :])
                bcp2 = bcps.tile([P, QS], F32, tag="bc", name=f"bcp2_{s}")
                nc.tensor.matmul(
                    bcp2[:], ones_row_1[:], l2_sb[:], start=True, stop=True
                )
                bt = bcsbp.tile([P, QS], F32, tag="bt", name=f"bt_{s}")
                nc.vector.reciprocal(bt[:], bcp2[:])

                conv_s = s + 1 if s + 1 < NS else None
                sa_s = s + 2 if s + 2 < NS else None
                if conv_s is not None:
                    cbt = c_chain(conv_s, la[conv_s])
                    a8[conv_s] = ([], [])
                if sa_s is not None:
                    la[sa_s] = new_la(sa_s)
                    ets[sa_s] = []
                    if sa_s % 2 == 0:
                        qp_n = proj_dma(xq, sa_s // 2, xqinp)

                for ctg in range(CT // VL_CTG):
                    vlt = vlp.tile([P, PT, VL_CTG * P], FP8, tag="vl",
                                   name=f"vl_{s}_{ctg}")
                    nc.sync.dma_start(
                        vlt[:],
                        vl_r[:, :, ctg * VL_CTG * P:(ctg + 1) * VL_CTG * P],
                    )
                    for ci in range(VL_CTG):
                        ct = ctg * VL_CTG + ci
                        up = ups.tile([P, QS], F32, tag="u", name=f"up_{s}_{ct}")
                        for t in range(NPAIR):
                            nc.tensor.matmul(
                                up[:],
                                vth[:, 2 * t:2 * t + 2, ct * P:(ct + 1) * P],
                                ah_t[t][:],
                                start=(t == 0), stop=False, perf_mode=DR,
                            )
                        for t in range(NPAIR):
                            nc.tensor.matmul(
                                up[:],
                                vlt[:, 2 * t:2 * t + 2, ci * P:(ci + 1) * P],
                                ah_t[t][:],
                                start=False, stop=False, perf_mode=DR,
                            )
                        for t in range(NPAIR):
                            nc.tensor.matmul(
                                up[:],
                                vth[:, 2 * t:2 * t + 2, ct * P:(ct + 1) * P],
                                al_t[t][:],
                                start=False, stop=(t == NPAIR - 1), perf_mode=DR,
                            )
                        osb = osbp.tile([P, QS], F32, tag="osb",
                                        name=f"osb_{s}_{ct}")
                        nc.vector.tensor_mul(out=osb[:], in0=up[:], in1=bt[:])
                        nc.sync.dma_start(
                            out[ct * P:(ct + 1) * P, qs:qs + QS], osb[:]
                        )
                        if conv_s is not None and 1 <= ct <= NPAIR - 1:
                            ah, alo = conv_pair(conv_s, ct - 1, ets[conv_s], cbt)
                            a8[conv_s][0].append(ah)
                            a8[conv_s][1].append(alo)
                        if sa_s is not None:
                            if sa_s % 2 == 0 and ct == 1:
                                proj_mms(qproj, sa_s // 2, qp_n)
                            if 3 <= ct <= 15:
                                for pt in (2 * (ct - 3), 2 * (ct - 3) + 1):
                                    ets[sa_s].append(
                                        stage_a_tile(pt, sa_s, la[sa_s])
                                    )
                if conv_s is not None:
                    ah, alo = conv_pair(conv_s, NPAIR - 1, ets[conv_s], cbt)
                    a8[conv_s][0].append(ah)
                    a8[conv_s][1].append(alo)
                if sa_s is not None:
                    for pt in range(2 * (NPAIR - 3), PT):
                        ets[sa_s].append(stage_a_tile(pt, sa_s, la[sa_s]))
                del a8[s]
                if s in ets:
                    del ets[s]
    nc.finalize()
    return nc


def _get_nc():
    if "nc" not in _CACHE:
        _CACHE["nc"] = _build()
    return _CACHE["nc"]


def _make_in_maps(query_features, prompt_features, W, b):
    qf = np.asarray(query_features, dtype=np.float32)
    pf = np.asarray(prompt_features, dtype=np.float32)
    Wm = np.asarray(W, dtype=np.float32)
    bv = np.asarray(b, dtype=np.float32)

    f8 = ml_dtypes.float8_e4m3
    wt = np.ascontiguousarray(Wm.T).astype(np.float16)
    xps, vhs, vls = [], [], []
    for bi in range(B):
        xpb = np.ascontiguousarray(pf[bi].reshape(C, HW))
        xps.append(xpb.astype(np.float16))
        vt = np.ascontiguousarray(xpb.T)
        vh8 = vt.astype(f8)
        vl8 = (vt - vh8.astype(np.float32)).astype(f8)
        vhs.append(vh8)
        vls.append(vl8)
    in_maps = []
    for core in range(NCORES):
        bi, h = divmod(core, 2)
        xqc = np.ascontiguousarray(
            qf[bi].reshape(C, HW)[:, h * QH:(h + 1) * QH]
        ).astype(np.float16)
        in_maps.append(
            {"xq": xqc, "xp": xps[bi], "vh": vhs[bi], "vl": vls[bi],
             "wt": wt, "bias": bv}
        )
    return in_maps


def _assemble(results):
    full = np.empty((B, C, HW), np.float32)
    for core in range(NCORES):
        bi, h = divmod(core, 2)
        full[bi][:, h * QH:(h + 1) * QH] = results[core]["out"]
    return full.reshape(B, C, H, Wd)


def kernel(query_features, prompt_features, W, b):
    nc = _get_nc()
    in_maps = _make_in_maps(query_features, prompt_features, W, b)
    last_err = None
    for _ in range(3):
        try:
            res = run_bass_kernel_spmd(nc, in_maps, list(range(NCORES)))
            return _assemble(res.results)
        except Exception as e:
            last_err = e
    raise last_err


def kernel_traced(query_features, prompt_features, W, b, **trace_kwargs):
    nc = _get_nc()
    in_maps = _make_in_maps(query_features, prompt_features, W, b)
    res = run_bass_kernel_spmd(
        nc, in_maps, list(range(NCORES)), trace=True, **trace_kwargs
    )
    return _assemble(res.results), res
